# revision 11
# baseline (speedup 1.0000x reference)
"""Trainium2 Bass kernel for nn_MixtureModelDecoder.

Model (see reference):
  atom classifier: h = z@W1+b1; BatchNorm(train stats over all atoms); GELU; @W2+b2
  edge head: logits[e,t] = z[i_e]^T Wsym_t z[j_e], softmax over t (T=5)

Sharding: 1024 molecules (32 atoms each) are sharded 128-per-core across 8
cores; a core's atom slab [4096,64] is exactly its molecule slab.  edge_index
from setup_inputs() is the fixed block structure "all C(32,2) pairs within
each molecule", so the edge head is computed as per-molecule Gram-type
matrices G_t = Z_m Wsym_t Z_m^T (no gather at all); softmax is done on-chip
over all 32x32 pairs and the host extracts the upper triangle.  BatchNorm
batch stats are computed analytically from C = z^T z and sum(z) accumulated
on-device over the full (replicated) z, so no cross-core collective is
needed.
"""

import sys

sys.path.insert(0, "/opt/trn_rl_repo")

import numpy as np

import concourse.bass as bass
import concourse.mybir as mybir
import concourse.tile as tile
from concourse import bacc
from concourse.bass_utils import run_bass_kernel_spmd

N_CORES = 8
A = 32768  # total atoms
D = 64  # z dim
H = 256  # hidden
T = 5  # bond types
NT = 10  # atom types
NMOL = 1024
NA = 32  # atoms per molecule
AC = A // N_CORES  # atoms per core (4096)
MC = NMOL // N_CORES  # molecules per core (128)
BN_EPS = 1e-5
F32 = mybir.dt.float32

LAST_RESULT = None  # BassKernelResults of the most recent device run


def _ap(src, ap_list):
    return bass.AP(tensor=src.tensor, offset=src.offset, ap=ap_list)


def _build_bass():
    nc = bacc.Bacc(None, target_bir_lowering=False, debug=False)

    z_aug = nc.dram_tensor("z_aug", [A, D + 1], F32, kind="ExternalInput")
    z_loct = nc.dram_tensor("z_loct", [D, AC], F32, kind="ExternalInput")
    w1 = nc.dram_tensor("w1", [D, H], F32, kind="ExternalInput")
    b1 = nc.dram_tensor("b1", [H], F32, kind="ExternalInput")
    gam = nc.dram_tensor("gam", [H], F32, kind="ExternalInput")
    bet = nc.dram_tensor("bet", [H], F32, kind="ExternalInput")
    w2 = nc.dram_tensor("w2", [H, NT], F32, kind="ExternalInput")
    b2 = nc.dram_tensor("b2", [NT], F32, kind="ExternalInput")
    wsym = nc.dram_tensor("wsym", [T, D, D], F32, kind="ExternalInput")

    at_out = nc.dram_tensor("atom_types_t", [NT, AC], F32, kind="ExternalOutput")
    eg_out = nc.dram_tensor("edge_soft", [MC, NA, T, NA], F32, kind="ExternalOutput")

    AX = mybir.AxisListType
    OP = mybir.AluOpType
    AF = mybir.ActivationFunctionType

    with tile.TileContext(nc) as tc:
        with (
            tc.tile_pool(name="consts", bufs=1) as consts,
            tc.tile_pool(name="zstream", bufs=16) as zstream,
            tc.tile_pool(name="hnpool", bufs=3) as hnpool,
            tc.tile_pool(name="smax", bufs=3) as smax,
            tc.tile_pool(name="outp", bufs=3) as outp,
            tc.tile_pool(name="ps_acc", bufs=1, space="PSUM") as ps_acc,
            tc.tile_pool(name="ps_big", bufs=3, space="PSUM") as ps_big,
            tc.tile_pool(name="ps_vp", bufs=2, space="PSUM") as ps_vp,
            tc.tile_pool(name="ps_small", bufs=2, space="PSUM") as ps_small,
        ):
            # ---------------- constants ----------------
            ones_sb = consts.tile([128, 1], F32, tag="ones")
            nc.vector.memset(ones_sb, 1.0)
            eps_sb = consts.tile([128, 1], F32, tag="eps")
            nc.vector.memset(eps_sb, BN_EPS)

            w1_sb = consts.tile([D, H], F32, tag="w1")
            nc.sync.dma_start(out=w1_sb, in_=w1[:, :])
            wsym_sb = consts.tile([D, T, D], F32, tag="wsym")
            nc.sync.dma_start(out=wsym_sb, in_=wsym[:, :, :].rearrange("t d c -> d t c"))
            w2_sb = consts.tile([128, 2, NT], F32, tag="w2")
            nc.sync.dma_start(
                out=w2_sb, in_=w2[:, :].rearrange("(h p) n -> p h n", p=128)
            )
            b1t_sb = consts.tile([128, 2], F32, tag="b1t")
            nc.sync.dma_start(out=b1t_sb, in_=b1[:].rearrange("(h p) -> p h", p=128))
            gamt_sb = consts.tile([128, 2], F32, tag="gamt")
            nc.sync.dma_start(out=gamt_sb, in_=gam[:].rearrange("(h p) -> p h", p=128))
            bett_sb = consts.tile([128, 2], F32, tag="bett")
            nc.sync.dma_start(out=bett_sb, in_=bet[:].rearrange("(h p) -> p h", p=128))
            b2t_sb = consts.tile([NT, 1], F32, tag="b2t")
            nc.sync.dma_start(out=b2t_sb, in_=b2[:].rearrange("(p o) -> p o", o=1))

            # persistent big sbuf tensors
            z_ct = consts.tile([D, AC], F32, tag="zct")  # z_loc^T
            v_sb = consts.tile([D, T, AC], F32, tag="vsb")  # Wsym_t @ z_ct

            # ---------------- phase A2: load pre-transposed local z --------
            nc.sync.dma_start(out=z_ct, in_=z_loct[:, :])

            # ---------------- phase D1: V_t = Wsym_t @ z_ct ----------------
            for t in range(T):
                for k in range(AC // 512):
                    vp = ps_vp.tile([D, 512], F32, tag="vp")
                    nc.tensor.matmul(
                        vp, lhsT=wsym_sb[:, t, :], rhs=z_ct[:, k * 512 : (k + 1) * 512]
                    )
                    nc.scalar.copy(
                        out=v_sb[:, t, k * 512 : (k + 1) * 512], in_=vp
                    )

            # ---------------- phase A: global z stats C=z^T z, s=z^T 1 -----
            cs_ps = ps_acc.tile([D, D + 1], F32, tag="cs")
            n_big = 16
            rows_per = A // n_big  # 2048 rows
            sub_per = rows_per // 128  # 16 subtiles
            for c in range(n_big):
                zt = zstream.tile([128, sub_per, D + 1], F32, tag="zt")
                nc.sync.dma_start(
                    out=zt,
                    in_=z_aug[c * rows_per : (c + 1) * rows_per, :].rearrange(
                        "(s p) d -> p s d", p=128
                    ),
                )
                for s in range(sub_per):
                    nc.tensor.matmul(
                        cs_ps,
                        lhsT=zt[:, s, :D],
                        rhs=zt[:, s, :],
                        start=(c == 0 and s == 0),
                        stop=(c == n_big - 1 and s == sub_per - 1),
                    )

            # ---------------- phase B: BN scale/shift ----------------
            cs_sb = smax.tile([D, D + 1], F32, tag="cs_sb")
            nc.scalar.copy(out=cs_sb, in_=cs_ps)
            m_sb = smax.tile([D, 1], F32, tag="m_sb")
            nc.scalar.mul(out=m_sb, in_=cs_sb[:, D : D + 1], mul=1.0 / A)
            cw_ps = ps_small.tile([D, H], F32, tag="ps")
            nc.tensor.matmul(cw_ps, lhsT=cs_sb[:, :D], rhs=w1_sb)
            wcw_sb = smax.tile([D, H], F32, tag="wcw")
            nc.vector.scalar_tensor_tensor(
                out=wcw_sb,
                in0=cw_ps,
                scalar=1.0 / A,
                in1=w1_sb,
                op0=OP.mult,
                op1=OP.mult,
            )
            st4 = ps_small.tile([128, 4], F32, tag="ps")
            for h in range(2):
                # E[(z.w)^2] per hidden unit (column h*128..)
                nc.tensor.matmul(
                    st4[:, h : h + 1],
                    lhsT=wcw_sb[:, h * 128 : (h + 1) * 128],
                    rhs=ones_sb[:D, :],
                )
                # mean(z).w
                nc.tensor.matmul(
                    st4[:, 2 + h : 3 + h],
                    lhsT=w1_sb[:, h * 128 : (h + 1) * 128],
                    rhs=m_sb,
                )
            st4_sb = smax.tile([128, 4], F32, tag="st4_sb")
            nc.scalar.copy(out=st4_sb, in_=st4)
            mh2 = smax.tile([128, 2], F32, tag="mh2")
            nc.vector.tensor_mul(out=mh2, in0=st4_sb[:, 2:4], in1=st4_sb[:, 2:4])
            var2 = smax.tile([128, 2], F32, tag="var2")
            nc.vector.tensor_sub(out=var2, in0=st4_sb[:, 0:2], in1=mh2)
            sd2 = smax.tile([128, 2], F32, tag="sd2")
            nc.scalar.activation(out=sd2, in_=var2, func=AF.Sqrt, bias=eps_sb)
            rstd2 = smax.tile([128, 2], F32, tag="rstd2")
            nc.vector.reciprocal(out=rstd2, in_=sd2)
            scale_sb = consts.tile([128, 2], F32, tag="scale")
            nc.vector.tensor_mul(out=scale_sb, in0=rstd2, in1=gamt_sb)
            meanh = smax.tile([128, 2], F32, tag="meanh")
            nc.vector.tensor_add(out=meanh, in0=st4_sb[:, 2:4], in1=b1t_sb)
            msc = smax.tile([128, 2], F32, tag="msc")
            nc.vector.tensor_mul(out=msc, in0=meanh, in1=scale_sb)
            shift_sb = consts.tile([128, 2], F32, tag="shift")
            nc.vector.tensor_sub(out=shift_sb, in0=bett_sb, in1=msc)

            # ---------------- phase C: classifier ----------------
            for k in range(AC // 512):
                hns = []
                for h in range(2):
                    htp = ps_big.tile([128, 512], F32, tag="big")
                    nc.tensor.matmul(
                        htp,
                        lhsT=w1_sb[:, h * 128 : (h + 1) * 128],
                        rhs=z_ct[:, k * 512 : (k + 1) * 512],
                    )
                    hn = hnpool.tile([128, 512], F32, tag="hn")
                    nc.scalar.activation(
                        out=hn,
                        in_=htp,
                        func=AF.Gelu,
                        bias=shift_sb[:, h : h + 1],
                        scale=scale_sb[:, h : h + 1],
                    )
                    hns.append(hn)
                atp = ps_small.tile([NT, 512], F32, tag="ps")
                nc.tensor.matmul(
                    atp, lhsT=w2_sb[:, 0, :], rhs=hns[0], start=True, stop=False
                )
                nc.tensor.matmul(
                    atp, lhsT=w2_sb[:, 1, :], rhs=hns[1], start=False, stop=True
                )
                ats = outp.tile([NT, 512], F32, tag="ats")
                nc.scalar.activation(out=ats, in_=atp, func=AF.Identity, bias=b2t_sb)
                nc.sync.dma_start(
                    out=at_out[:, k * 512 : (k + 1) * 512], in_=ats
                )

            # ---------------- phase D2: per-molecule Gram + softmax --------
            # PE matmul PSUM writes must start at partition 0/32/64, so pack
            # 3 molecules per [96, T*NA] tile.
            eg_flat = eg_out[:, :, :, :].rearrange("m a t b -> (m a) (t b)")
            groups = []
            m0 = 0
            while m0 < MC:
                nm = min(3, MC - m0)
                groups.append((m0, nm))
                m0 += nm
            for m0, nm in groups:
                P = nm * NA
                gp = ps_small.tile([128, T * NA], F32, tag="ps")
                for mg in range(nm):
                    a0 = (m0 + mg) * NA
                    for t in range(T):
                        nc.tensor.matmul(
                            gp[mg * NA : (mg + 1) * NA, t * NA : (t + 1) * NA],
                            lhsT=z_ct[:, a0 : a0 + NA],
                            rhs=v_sb[:, t, a0 : a0 + NA],
                        )
                # softmax over t within each (row, a2) pair
                gps = gp[:P, :]
                # t-innermost strided views ([p][a2][t])
                gp_ti = _ap(gps, [gps.ap[0], [1, NA], [NA, T]])
                mx = smax.tile([128, NA], F32, tag="mx")
                nc.vector.tensor_reduce(
                    out=mx[:P, :], in_=gp_ti, axis=AX.X, op=OP.max
                )
                mxs = mx[:P, :]
                e_sb = smax.tile([128, T * NA], F32, tag="e_sb")
                es = e_sb[:P, :]
                e_nat = _ap(es, [es.ap[0], [NA, T], [1, NA]])
                gp_nat = _ap(gps, [gps.ap[0], [NA, T], [1, NA]])
                mx_bc = _ap(mxs, [mxs.ap[0], [0, T], [1, NA]])
                nc.vector.tensor_sub(out=e_nat, in0=gp_nat, in1=mx_bc)
                nc.scalar.activation(out=es, in_=es, func=AF.Exp)
                e_ti = _ap(es, [es.ap[0], [1, NA], [NA, T]])
                sm = smax.tile([128, NA], F32, tag="sm")
                nc.vector.tensor_reduce(
                    out=sm[:P, :], in_=e_ti, axis=AX.X, op=OP.add
                )
                rc = smax.tile([128, NA], F32, tag="rc")
                nc.vector.reciprocal(out=rc[:P, :], in_=sm[:P, :])
                rcs = rc[:P, :]
                o_sb = outp.tile([128, T * NA], F32, tag="o_sb")
                os_ = o_sb[:P, :]
                o_nat = _ap(os_, [os_.ap[0], [NA, T], [1, NA]])
                rc_bc = _ap(rcs, [rcs.ap[0], [0, T], [1, NA]])
                nc.vector.tensor_mul(out=o_nat, in0=e_nat, in1=rc_bc)
                nc.sync.dma_start(
                    out=eg_flat[m0 * NA : m0 * NA + P, :], in_=os_
                )

    nc.compile()
    return nc


_NC_CACHE = None


def _get_nc():
    global _NC_CACHE
    if _NC_CACHE is None:
        _NC_CACHE = _build_bass()
    return _NC_CACHE


def _edge_structure_ok(edge_index):
    import itertools

    base = np.array(list(itertools.combinations(range(NA), 2)), dtype=np.int64)
    offs = (NA * np.arange(NMOL, dtype=np.int64))[:, None, None]
    exp = (base[None] + offs).reshape(-1, 2)
    ei = np.asarray(edge_index, dtype=np.int64)
    return ei.shape == exp.shape and np.array_equal(ei, exp)


def _edge_fallback_numpy(z, wsym, edge_index):
    ei = np.asarray(edge_index, dtype=np.int64)
    z1 = z[ei[:, 0]]
    z2 = z[ei[:, 1]]
    # logits[e,t] = sum_c (z1 @ Wsym_t)[e,c] * z2[e,c]
    u = z1 @ wsym.transpose(1, 0, 2).reshape(D, T * D)  # [E, T*D]
    logits = np.einsum("etc,ec->et", u.reshape(-1, T, D), z2)
    logits -= logits.max(axis=1, keepdims=True)
    np.exp(logits, out=logits)
    logits /= logits.sum(axis=1, keepdims=True)
    return logits


def kernel(z, W1, b1, bn_gamma, bn_beta, W2, b2, bond_matrix, edge_index):
    global LAST_RESULT
    z = np.ascontiguousarray(np.asarray(z, dtype=np.float32))
    W1 = np.ascontiguousarray(np.asarray(W1, dtype=np.float32))
    b1 = np.ascontiguousarray(np.asarray(b1, dtype=np.float32))
    bn_gamma = np.ascontiguousarray(np.asarray(bn_gamma, dtype=np.float32))
    bn_beta = np.ascontiguousarray(np.asarray(bn_beta, dtype=np.float32))
    W2 = np.ascontiguousarray(np.asarray(W2, dtype=np.float32))
    b2 = np.ascontiguousarray(np.asarray(b2, dtype=np.float32))
    bm = np.asarray(bond_matrix, dtype=np.float32)
    wsym = np.ascontiguousarray(0.5 * (bm + bm.transpose(0, 2, 1)))

    z_aug = np.ascontiguousarray(
        np.concatenate([z, np.ones((A, 1), np.float32)], axis=1)
    )

    nc = _get_nc()
    in_maps = []
    for c in range(N_CORES):
        in_maps.append(
            {
                "z_aug": z_aug,
                "z_loct": np.ascontiguousarray(z[c * AC : (c + 1) * AC].T),
                "w1": W1,
                "b1": b1,
                "gam": bn_gamma,
                "bet": bn_beta,
                "w2": W2,
                "b2": b2,
                "wsym": wsym,
            }
        )

    LAST_RESULT = run_bass_kernel_spmd(nc, in_maps, list(range(N_CORES)))
    results = LAST_RESULT.results

    atom_types = np.concatenate(
        [np.ascontiguousarray(r["atom_types_t"].T) for r in results], axis=0
    )

    if _edge_structure_ok(edge_index):
        ef = np.concatenate([r["edge_soft"] for r in results], axis=0)
        iu0, iu1 = np.triu_indices(NA, 1)
        # ef[m, i, t, j] for pairs (i<j); advanced idx at axes 1,3 -> [496, NMOL, T]
        edge_types = (
            ef[:, iu0, :, iu1].transpose(1, 0, 2).reshape(-1, T)
        )
        edge_types = np.ascontiguousarray(edge_types)
    else:
        sys.stderr.write(
            "kernel.py: edge_index does not match the expected molecular "
            "block structure; computing edge head on host as fallback\n"
        )
        edge_types = _edge_fallback_numpy(z, wsym, edge_index)

    return atom_types, edge_types


# revision 12
# speedup vs baseline: 1.2374x; 1.2374x over previous
"""Trainium2 Bass kernel for nn_MixtureModelDecoder.

Model (see reference):
  atom classifier: h = z@W1+b1; BatchNorm(train stats over all atoms); GELU; @W2+b2
  edge head: logits[e,t] = z[i_e]^T Wsym_t z[j_e], softmax over t (T=5)

Sharding: 1024 molecules (32 atoms each) are sharded 128-per-core across 8
cores; a core's atom slab [4096,64] is exactly its molecule slab.  edge_index
from setup_inputs() is the fixed block structure "all C(32,2) pairs within
each molecule", so the edge head is computed as per-molecule Gram-type
matrices G_t = Z_m Wsym_t Z_m^T (no gather at all); softmax is done on-chip
over all 32x32 pairs and the host extracts the upper triangle.  BatchNorm
batch stats are computed analytically from C = z^T z and sum(z) accumulated
on-device over the full (replicated) z, so no cross-core collective is
needed.
"""

import sys

sys.path.insert(0, "/opt/trn_rl_repo")

import numpy as np

import concourse.bass as bass
import concourse.mybir as mybir
import concourse.tile as tile
from concourse import bacc
from concourse.bass_utils import run_bass_kernel_spmd

N_CORES = 8
A = 32768  # total atoms
D = 64  # z dim
H = 256  # hidden
T = 5  # bond types
NT = 10  # atom types
NMOL = 1024
NA = 32  # atoms per molecule
AC = A // N_CORES  # atoms per core (4096)
MC = NMOL // N_CORES  # molecules per core (128)
BN_EPS = 1e-5
F32 = mybir.dt.float32

LAST_RESULT = None  # BassKernelResults of the most recent device run


def _ap(src, ap_list):
    return bass.AP(tensor=src.tensor, offset=src.offset, ap=ap_list)


def _build_bass():
    nc = bacc.Bacc(None, target_bir_lowering=False, debug=False)

    z_aug = nc.dram_tensor("z_aug", [A, D + 1], F32, kind="ExternalInput")
    z_loct = nc.dram_tensor("z_loct", [D, AC], F32, kind="ExternalInput")
    w1 = nc.dram_tensor("w1", [D, H], F32, kind="ExternalInput")
    b1 = nc.dram_tensor("b1", [H], F32, kind="ExternalInput")
    gam = nc.dram_tensor("gam", [H], F32, kind="ExternalInput")
    bet = nc.dram_tensor("bet", [H], F32, kind="ExternalInput")
    w2 = nc.dram_tensor("w2", [H, NT], F32, kind="ExternalInput")
    b2 = nc.dram_tensor("b2", [NT], F32, kind="ExternalInput")
    wsym = nc.dram_tensor("wsym", [T, D, D], F32, kind="ExternalInput")

    at_out = nc.dram_tensor("atom_types_t", [NT, AC], F32, kind="ExternalOutput")
    eg_out = nc.dram_tensor("edge_soft", [MC, NA, T, NA], F32, kind="ExternalOutput")

    AX = mybir.AxisListType
    OP = mybir.AluOpType
    AF = mybir.ActivationFunctionType

    with tile.TileContext(nc) as tc:
        with (
            tc.tile_pool(name="consts", bufs=1) as consts,
            tc.tile_pool(name="zstream", bufs=16) as zstream,
            tc.tile_pool(name="hnpool", bufs=3) as hnpool,
            tc.tile_pool(name="smax", bufs=3) as smax,
            tc.tile_pool(name="outp", bufs=3) as outp,
            tc.tile_pool(name="ps_acc", bufs=1, space="PSUM") as ps_acc,
            tc.tile_pool(name="ps_big", bufs=3, space="PSUM") as ps_big,
            tc.tile_pool(name="ps_vp", bufs=2, space="PSUM") as ps_vp,
            tc.tile_pool(name="ps_small", bufs=2, space="PSUM") as ps_small,
        ):
            # ---------------- constants ----------------
            ones_sb = consts.tile([128, 1], F32, tag="ones")
            nc.vector.memset(ones_sb, 1.0)
            eps_sb = consts.tile([128, 1], F32, tag="eps")
            nc.vector.memset(eps_sb, BN_EPS)

            w1_sb = consts.tile([D, H], F32, tag="w1")
            nc.sync.dma_start(out=w1_sb, in_=w1[:, :])
            wsym_sb = consts.tile([D, T, D], F32, tag="wsym")
            nc.sync.dma_start(out=wsym_sb, in_=wsym[:, :, :].rearrange("t d c -> d t c"))
            w2_sb = consts.tile([128, 2, NT], F32, tag="w2")
            nc.sync.dma_start(
                out=w2_sb, in_=w2[:, :].rearrange("(h p) n -> p h n", p=128)
            )
            b1t_sb = consts.tile([128, 2], F32, tag="b1t")
            nc.sync.dma_start(out=b1t_sb, in_=b1[:].rearrange("(h p) -> p h", p=128))
            gamt_sb = consts.tile([128, 2], F32, tag="gamt")
            nc.sync.dma_start(out=gamt_sb, in_=gam[:].rearrange("(h p) -> p h", p=128))
            bett_sb = consts.tile([128, 2], F32, tag="bett")
            nc.sync.dma_start(out=bett_sb, in_=bet[:].rearrange("(h p) -> p h", p=128))
            b2t_sb = consts.tile([NT, 1], F32, tag="b2t")
            nc.sync.dma_start(out=b2t_sb, in_=b2[:].rearrange("(p o) -> p o", o=1))

            # persistent big sbuf tensors
            z_ct = consts.tile([D, AC], F32, tag="zct")  # z_loc^T
            v_sb = consts.tile([D, T, AC], F32, tag="vsb")  # Wsym_t @ z_ct

            # ---------------- phase A2: load pre-transposed local z --------
            nc.sync.dma_start(out=z_ct, in_=z_loct[:, :])

            # ---------------- phase D1: V_t = Wsym_t @ z_ct ----------------
            for t in range(T):
                for k in range(AC // 512):
                    vp = ps_vp.tile([D, 512], F32, tag="vp")
                    nc.tensor.matmul(
                        vp, lhsT=wsym_sb[:, t, :], rhs=z_ct[:, k * 512 : (k + 1) * 512]
                    )
                    if (t * 8 + k) % 2 == 0:
                        nc.vector.tensor_copy(
                            out=v_sb[:, t, k * 512 : (k + 1) * 512], in_=vp
                        )
                    else:
                        nc.scalar.copy(
                            out=v_sb[:, t, k * 512 : (k + 1) * 512], in_=vp
                        )

            # ---------------- phase A: global z stats C=z^T z, s=z^T 1 -----
            cs_ps = ps_acc.tile([D, D + 1], F32, tag="cs")
            n_big = 16
            rows_per = A // n_big  # 2048 rows
            sub_per = rows_per // 128  # 16 subtiles
            for c in range(n_big):
                zt = zstream.tile([128, sub_per, D + 1], F32, tag="zt")
                nc.sync.dma_start(
                    out=zt,
                    in_=z_aug[c * rows_per : (c + 1) * rows_per, :].rearrange(
                        "(s p) d -> p s d", p=128
                    ),
                )
                for s in range(sub_per):
                    nc.tensor.matmul(
                        cs_ps,
                        lhsT=zt[:, s, :D],
                        rhs=zt[:, s, :],
                        start=(c == 0 and s == 0),
                        stop=(c == n_big - 1 and s == sub_per - 1),
                    )

            # ---------------- phase B: BN scale/shift ----------------
            cs_sb = smax.tile([D, D + 1], F32, tag="cs_sb")
            nc.scalar.copy(out=cs_sb, in_=cs_ps)
            m_sb = smax.tile([D, 1], F32, tag="m_sb")
            nc.scalar.mul(out=m_sb, in_=cs_sb[:, D : D + 1], mul=1.0 / A)
            cw_ps = ps_small.tile([D, H], F32, tag="ps")
            nc.tensor.matmul(cw_ps, lhsT=cs_sb[:, :D], rhs=w1_sb)
            wcw_sb = smax.tile([D, H], F32, tag="wcw")
            nc.vector.scalar_tensor_tensor(
                out=wcw_sb,
                in0=cw_ps,
                scalar=1.0 / A,
                in1=w1_sb,
                op0=OP.mult,
                op1=OP.mult,
            )
            st4 = ps_small.tile([128, 4], F32, tag="ps")
            for h in range(2):
                # E[(z.w)^2] per hidden unit (column h*128..)
                nc.tensor.matmul(
                    st4[:, h : h + 1],
                    lhsT=wcw_sb[:, h * 128 : (h + 1) * 128],
                    rhs=ones_sb[:D, :],
                )
                # mean(z).w
                nc.tensor.matmul(
                    st4[:, 2 + h : 3 + h],
                    lhsT=w1_sb[:, h * 128 : (h + 1) * 128],
                    rhs=m_sb,
                )
            st4_sb = smax.tile([128, 4], F32, tag="st4_sb")
            nc.scalar.copy(out=st4_sb, in_=st4)
            mh2 = smax.tile([128, 2], F32, tag="mh2")
            nc.vector.tensor_mul(out=mh2, in0=st4_sb[:, 2:4], in1=st4_sb[:, 2:4])
            var2 = smax.tile([128, 2], F32, tag="var2")
            nc.vector.tensor_sub(out=var2, in0=st4_sb[:, 0:2], in1=mh2)
            sd2 = smax.tile([128, 2], F32, tag="sd2")
            nc.scalar.activation(out=sd2, in_=var2, func=AF.Sqrt, bias=eps_sb)
            rstd2 = smax.tile([128, 2], F32, tag="rstd2")
            nc.vector.reciprocal(out=rstd2, in_=sd2)
            scale_sb = consts.tile([128, 2], F32, tag="scale")
            nc.vector.tensor_mul(out=scale_sb, in0=rstd2, in1=gamt_sb)
            meanh = smax.tile([128, 2], F32, tag="meanh")
            nc.vector.tensor_add(out=meanh, in0=st4_sb[:, 2:4], in1=b1t_sb)
            msc = smax.tile([128, 2], F32, tag="msc")
            nc.vector.tensor_mul(out=msc, in0=meanh, in1=scale_sb)
            shift_sb = consts.tile([128, 2], F32, tag="shift")
            nc.vector.tensor_sub(out=shift_sb, in0=bett_sb, in1=msc)

            # ---------------- phase C: classifier ----------------
            for k in range(AC // 512):
                hns = []
                for h in range(2):
                    htp = ps_big.tile([128, 512], F32, tag="big")
                    nc.tensor.matmul(
                        htp,
                        lhsT=w1_sb[:, h * 128 : (h + 1) * 128],
                        rhs=z_ct[:, k * 512 : (k + 1) * 512],
                    )
                    hn = hnpool.tile([128, 512], F32, tag="hn")
                    nc.scalar.activation(
                        out=hn,
                        in_=htp,
                        func=AF.Gelu,
                        bias=shift_sb[:, h : h + 1],
                        scale=scale_sb[:, h : h + 1],
                    )
                    hns.append(hn)
                atp = ps_small.tile([NT, 512], F32, tag="ps")
                nc.tensor.matmul(
                    atp, lhsT=w2_sb[:, 0, :], rhs=hns[0], start=True, stop=False
                )
                nc.tensor.matmul(
                    atp, lhsT=w2_sb[:, 1, :], rhs=hns[1], start=False, stop=True
                )
                ats = outp.tile([NT, 512], F32, tag="ats")
                nc.scalar.activation(out=ats, in_=atp, func=AF.Identity, bias=b2t_sb)
                nc.sync.dma_start(
                    out=at_out[:, k * 512 : (k + 1) * 512], in_=ats
                )

            # ---------------- phase D2: per-molecule Gram + softmax --------
            # PE matmul PSUM writes must start at partition 0/32/64, so pack
            # 3 molecules per [96, T*NA] tile.
            eg_flat = eg_out[:, :, :, :].rearrange("m a t b -> (m a) (t b)")
            groups = []
            m0 = 0
            while m0 < MC:
                nm = min(3, MC - m0)
                groups.append((m0, nm))
                m0 += nm
            for m0, nm in groups:
                P = nm * NA
                gp = ps_small.tile([128, T * NA], F32, tag="ps")
                for mg in range(nm):
                    a0 = (m0 + mg) * NA
                    for t in range(T):
                        nc.tensor.matmul(
                            gp[mg * NA : (mg + 1) * NA, t * NA : (t + 1) * NA],
                            lhsT=z_ct[:, a0 : a0 + NA],
                            rhs=v_sb[:, t, a0 : a0 + NA],
                        )
                # softmax over t within each (row, a2) pair
                gps = gp[:P, :]
                # t-innermost strided views ([p][a2][t])
                gp_ti = _ap(gps, [gps.ap[0], [1, NA], [NA, T]])
                mx = smax.tile([128, NA], F32, tag="mx")
                nc.vector.tensor_reduce(
                    out=mx[:P, :], in_=gp_ti, axis=AX.X, op=OP.max
                )
                mxs = mx[:P, :]
                e_sb = smax.tile([128, T * NA], F32, tag="e_sb")
                es = e_sb[:P, :]
                e_nat = _ap(es, [es.ap[0], [NA, T], [1, NA]])
                gp_nat = _ap(gps, [gps.ap[0], [NA, T], [1, NA]])
                mx_bc = _ap(mxs, [mxs.ap[0], [0, T], [1, NA]])
                nc.vector.tensor_sub(out=e_nat, in0=gp_nat, in1=mx_bc)
                nc.scalar.activation(out=es, in_=es, func=AF.Exp)
                e_ti = _ap(es, [es.ap[0], [1, NA], [NA, T]])
                sm = smax.tile([128, NA], F32, tag="sm")
                nc.vector.tensor_reduce(
                    out=sm[:P, :], in_=e_ti, axis=AX.X, op=OP.add
                )
                rc = smax.tile([128, NA], F32, tag="rc")
                nc.vector.reciprocal(out=rc[:P, :], in_=sm[:P, :])
                rcs = rc[:P, :]
                o_sb = outp.tile([128, T * NA], F32, tag="o_sb")
                os_ = o_sb[:P, :]
                o_nat = _ap(os_, [os_.ap[0], [NA, T], [1, NA]])
                rc_bc = _ap(rcs, [rcs.ap[0], [0, T], [1, NA]])
                nc.vector.tensor_mul(out=o_nat, in0=e_nat, in1=rc_bc)
                nc.sync.dma_start(
                    out=eg_flat[m0 * NA : m0 * NA + P, :], in_=os_
                )

    nc.compile()
    return nc


_NC_CACHE = None


def _get_nc():
    global _NC_CACHE
    if _NC_CACHE is None:
        _NC_CACHE = _build_bass()
    return _NC_CACHE


def _edge_structure_ok(edge_index):
    import itertools

    base = np.array(list(itertools.combinations(range(NA), 2)), dtype=np.int64)
    offs = (NA * np.arange(NMOL, dtype=np.int64))[:, None, None]
    exp = (base[None] + offs).reshape(-1, 2)
    ei = np.asarray(edge_index, dtype=np.int64)
    return ei.shape == exp.shape and np.array_equal(ei, exp)


def _edge_fallback_numpy(z, wsym, edge_index):
    ei = np.asarray(edge_index, dtype=np.int64)
    z1 = z[ei[:, 0]]
    z2 = z[ei[:, 1]]
    # logits[e,t] = sum_c (z1 @ Wsym_t)[e,c] * z2[e,c]
    u = z1 @ wsym.transpose(1, 0, 2).reshape(D, T * D)  # [E, T*D]
    logits = np.einsum("etc,ec->et", u.reshape(-1, T, D), z2)
    logits -= logits.max(axis=1, keepdims=True)
    np.exp(logits, out=logits)
    logits /= logits.sum(axis=1, keepdims=True)
    return logits


def kernel(z, W1, b1, bn_gamma, bn_beta, W2, b2, bond_matrix, edge_index):
    global LAST_RESULT
    z = np.ascontiguousarray(np.asarray(z, dtype=np.float32))
    W1 = np.ascontiguousarray(np.asarray(W1, dtype=np.float32))
    b1 = np.ascontiguousarray(np.asarray(b1, dtype=np.float32))
    bn_gamma = np.ascontiguousarray(np.asarray(bn_gamma, dtype=np.float32))
    bn_beta = np.ascontiguousarray(np.asarray(bn_beta, dtype=np.float32))
    W2 = np.ascontiguousarray(np.asarray(W2, dtype=np.float32))
    b2 = np.ascontiguousarray(np.asarray(b2, dtype=np.float32))
    bm = np.asarray(bond_matrix, dtype=np.float32)
    wsym = np.ascontiguousarray(0.5 * (bm + bm.transpose(0, 2, 1)))

    z_aug = np.ascontiguousarray(
        np.concatenate([z, np.ones((A, 1), np.float32)], axis=1)
    )

    nc = _get_nc()
    in_maps = []
    for c in range(N_CORES):
        in_maps.append(
            {
                "z_aug": z_aug,
                "z_loct": np.ascontiguousarray(z[c * AC : (c + 1) * AC].T),
                "w1": W1,
                "b1": b1,
                "gam": bn_gamma,
                "bet": bn_beta,
                "w2": W2,
                "b2": b2,
                "wsym": wsym,
            }
        )

    LAST_RESULT = run_bass_kernel_spmd(nc, in_maps, list(range(N_CORES)))
    results = LAST_RESULT.results

    atom_types = np.concatenate(
        [np.ascontiguousarray(r["atom_types_t"].T) for r in results], axis=0
    )

    if _edge_structure_ok(edge_index):
        ef = np.concatenate([r["edge_soft"] for r in results], axis=0)
        iu0, iu1 = np.triu_indices(NA, 1)
        # ef[m, i, t, j] for pairs (i<j); advanced idx at axes 1,3 -> [496, NMOL, T]
        edge_types = (
            ef[:, iu0, :, iu1].transpose(1, 0, 2).reshape(-1, T)
        )
        edge_types = np.ascontiguousarray(edge_types)
    else:
        sys.stderr.write(
            "kernel.py: edge_index does not match the expected molecular "
            "block structure; computing edge head on host as fallback\n"
        )
        edge_types = _edge_fallback_numpy(z, wsym, edge_index)

    return atom_types, edge_types
